# revision 56
# baseline (speedup 1.0000x reference)
"""Trainium2 Bass kernel: 16-head RoPE attention block (B=4, T=2048, D=2048).

Sharding: tensor-parallel over heads. Each of the 8 cores owns 2 heads
(a 256-wide slice of the q/k/v projection output features) and computes a
full-width partial of the output projection; the host sums the 8 fp16
partials (the "all-reduce").

v2 structure (vs the DRAM-scratch baseline):
  - everything flows in bf16 (weights, x, q/k/v, exp(S), attention, Wo);
    PSUM accumulation stays fp32. Halves DMA + SBUF and keeps the PE at
    1 col/cycle.
  - q/k/v live in SBUF per batch (double-buffered) - no DRAM round-trip.
  - per-batch software pipeline, riffled into one instruction stream so
    the PE queue interleaves stage-2 attention blocks of batch i with
    stage-1 projection chunks of batch i+1 and stage-3 out-projection
    blocks of batch i-1: the activation engine's exp backlog drains
    during the projection/out-proj blocks instead of stalling PV.
  - softmax denominators use a 128-wide ones matrix as the stationary
    operand, so the PE replicates sum(exp) across all partitions: the
    broadcast is free, and reciprocal+normalize fuse into two DVE ops
    per query chunk (no DRAM round-trip for the denominators).
  - outputs written as fp16 partials (halves the output DMA).
"""

import math

import numpy as np
import ml_dtypes

import concourse.bacc as bacc
import concourse.bass as bass
import concourse.mybir as mybir
import concourse.tile as tile
from concourse.bass_utils import run_bass_kernel_spmd

F32 = mybir.dt.float32
BF16 = mybir.dt.bfloat16
FP16 = mybir.dt.float16
EXP = mybir.ActivationFunctionType.Exp

# Problem shape (hardcoded; the harness calls kernel() with exactly these).
B = 4
T = 2048
D_MODEL = 2048
HEAD_DIM = 128
N_CORES = 8
ROPE_BASE = 10000.0

HPC = 2                      # heads per core
F_LOC = HPC * HEAD_DIM       # 256 local projection features per core
TCH = 512                    # stage-1 token chunk width
QCH = 512                    # stage-2 query chunk width
SCALE = 1.0 / math.sqrt(HEAD_DIM)
S_LOOK = 5                   # score-matmul lookahead in the attention loop


def build_module(b=B, t=T, d_model=D_MODEL, n_cores=N_CORES):
    """Build the per-core Bass module. All cores run the same program on
    different data (pure SPMD, no collectives)."""
    dt_ = d_model // 128     # 16 contraction tiles
    kt = t // 128            # 16 key tiles per batch
    cpb = t // TCH           # 4 stage-1 chunks per batch
    nqc = t // QCH           # 4 query chunks

    nc = bacc.Bacc(None, target_bir_lowering=False)

    xT = nc.dram_tensor("xT", [d_model, b * t], BF16, kind="ExternalInput")
    wqT = nc.dram_tensor("wqT", [d_model, F_LOC], BF16, kind="ExternalInput")
    wkT = nc.dram_tensor("wkT", [d_model, F_LOC], BF16, kind="ExternalInput")
    wvT = nc.dram_tensor("wvT", [d_model, F_LOC], BF16, kind="ExternalInput")
    woT = nc.dram_tensor("woT", [F_LOC, d_model], BF16, kind="ExternalInput")
    cosT = nc.dram_tensor("cosT", [HEAD_DIM, t], F32, kind="ExternalInput")
    rsinT = nc.dram_tensor("rsinT", [HEAD_DIM, t], F32, kind="ExternalInput")
    onesc = nc.dram_tensor("onesc", [128, 128], BF16, kind="ExternalInput")
    outP = nc.dram_tensor("outP", [d_model, b * t], FP16, kind="ExternalOutput")

    with tile.TileContext(nc) as tc:
        with (
            tc.tile_pool(name="const", bufs=1) as constp,
            tc.tile_pool(name="dram", bufs=1, space="DRAM") as dram,
            tc.tile_pool(name="wq", bufs=1) as wpool,
            tc.tile_pool(name="x", bufs=2) as xpool,
            tc.tile_pool(name="qkv", bufs=2) as qkvp,
            tc.tile_pool(name="t1", bufs=3) as tpool,
            tc.tile_pool(name="e", bufs=8) as epool,
            tc.tile_pool(name="pr", bufs=4) as prpool,
            tc.tile_pool(name="s2", bufs=2) as s2pool,
            tc.tile_pool(name="attn", bufs=2) as attnp,
            tc.tile_pool(name="s3o", bufs=2) as s3pool,
            tc.tile_pool(name="ps_mm", bufs=5, space="PSUM") as ps_mm,
            tc.tile_pool(name="ps_pv", bufs=2, space="PSUM") as ps_pv,
            tc.tile_pool(name="ps_dn", bufs=1, space="PSUM") as ps_dn,
        ):
            # ---- constants (gpsimd ring keeps sync/scalar free for the
            # weight/x loads that gate the first matmuls) ----
            cos_sb = constp.tile([128, t], BF16)
            nc.gpsimd.dma_start(out=cos_sb, in_=cosT[:, :])
            rsin_sb = constp.tile([128, t], BF16)
            nc.gpsimd.dma_start(out=rsin_sb, in_=rsinT[:, :])
            # 128-wide ones: the denominator matmul replicates sum(exp)
            # across all 128 output partitions, making the broadcast free
            ones_sb = constp.tile([128, 128], BF16)
            nc.gpsimd.dma_start(out=ones_sb, in_=onesc[:, :])

            # warm up the PE p-state while the weight/x DMAs land: short
            # matmuls on the ones tile, results never read
            warm_ps = ps_dn.tile([128, QCH], F32, tag="dn")
            for wu in range(36):
                nc.tensor.matmul(
                    warm_ps[:, 0:128], ones_sb, ones_sb, start=True, stop=True
                )

            # ---- weights ----
            w_sbs = []
            for wi, (wten, wname) in enumerate(
                ((wqT, "wq"), (wkT, "wk"), (wvT, "wv"))
            ):
                wsb = wpool.tile([128, dt_, F_LOC], BF16, tag=wname)
                src = wten[:, :].rearrange("(dt p) f -> p dt f", p=128)
                # wq on the sync ring ahead of x chunk 0, split so the
                # first projections start after 1/4 of the transfer;
                # wk/wv on the scalar HWDGE ring
                if wi == 0:
                    for d0 in range(0, dt_, 4):
                        nc.sync.dma_start(
                            out=wsb[:, d0 : d0 + 4, :],
                            in_=src[:, d0 : d0 + 4, :],
                        )
                else:
                    nc.scalar.dma_start(out=wsb, in_=src)
                w_sbs.append(wsb)
            wo_sb = wpool.tile([128, HPC, d_model], BF16, tag="wo")
            nc.scalar.dma_start(
                out=wo_sb,
                in_=woT[:, :].rearrange("(ft p) d -> p ft d", p=128),
            )

            # per-batch double-buffered SBUF state, created lazily
            qk_sb = {}       # (bi,) -> (q_sb, k_sb)  [128, HPC, t] bf16
            v_sb = {}        # (bi,) -> [128, kt, HPC, 128] bf16
            attn_sb = {}     # (bi,) -> [128, HPC, t] bf16

            # ================= emission units =========================
            x_tiles = {}

            def s1_load(bi, c):
                """Issue the x-chunk DMA (placed ahead of its compute)."""
                off = c * TCH
                tsl = slice(bi * t + off, bi * t + off + TCH)
                x_sb = xpool.tile([128, dt_, TCH], BF16, name=f"x{bi}_{c}", tag="x")
                xsrc = xT[:, tsl].rearrange("(dt p) tt -> p dt tt", p=128)
                if bi == 0 and c == 0:
                    # split the first chunk's load so the very first
                    # matmuls start after 1/4 of the transfer
                    for d0 in range(0, dt_, 4):
                        nc.sync.dma_start(
                            out=x_sb[:, d0 : d0 + 4, :],
                            in_=xsrc[:, d0 : d0 + 4, :],
                        )
                else:
                    nc.sync.dma_start(out=x_sb, in_=xsrc)
                x_tiles[(bi, c)] = x_sb

            def s1_chunk(bi, c):
                """Projections + rope + v-transpose for 512 tokens."""
                if c == 0:
                    qk_sb[bi] = (
                        qkvp.tile([128, HPC, t], BF16, name=f"q{bi}", tag="q"),
                        qkvp.tile([128, HPC, t], BF16, name=f"k{bi}", tag="k"),
                    )
                    v_sb[bi] = qkvp.tile(
                        [128, kt, HPC, 128], BF16, name=f"v{bi}", tag="v"
                    )
                off = c * TCH
                lsl = slice(off, off + TCH)
                x_sb = x_tiles.pop((bi, c))
                for pi in range(3):
                    for ft in range(HPC):
                        fsl = slice(ft * 128, (ft + 1) * 128)
                        ps = ps_mm.tile([128, TCH], F32, tag="mm")
                        for di in range(dt_):
                            nc.tensor.matmul(
                                ps,
                                w_sbs[pi][:, di, fsl],
                                x_sb[:, di, :],
                                start=(di == 0),
                                stop=(di == dt_ - 1),
                            )
                        if pi < 2:
                            # rope: out = in*cos + rot_half(in)*sin
                            ro = tpool.tile([128, TCH], F32, tag="ro")
                            nc.vector.tensor_mul(ro, ps, cos_sb[:, lsl])
                            rt = tpool.tile([128, TCH], F32, tag="rt")
                            nc.vector.tensor_mul(
                                rt[0:64], ps[64:128], rsin_sb[0:64, lsl]
                            )
                            nc.vector.tensor_mul(
                                rt[64:128], ps[0:64], rsin_sb[64:128, lsl]
                            )
                            dst = qk_sb[bi][pi]
                            nc.vector.tensor_add(dst[:, ft, lsl], ro, rt)
                        else:
                            # v lands token-major via an XBAR DMA transpose
                            # per chunk (out[p, j, d] = vsb[d, j*128+p])
                            vsb = tpool.tile([128, TCH], BF16, tag="vs")
                            nc.scalar.copy(vsb, ps)
                            j0 = c * (TCH // 128)
                            nc.sync.dma_start_transpose(
                                v_sb[bi][:, j0 : j0 + TCH // 128, ft, :], vsb
                            )

            def s2_block(bi, h, qc):
                """Attention for one (batch, head, 512-query chunk)."""
                if h == 0 and qc == 0:
                    attn_sb[bi] = attnp.tile(
                        [128, HPC, t], BF16, name=f"an{bi}", tag="an"
                    )
                q_t, k_t = qk_sb[bi]
                qsl = slice(qc * QCH, (qc + 1) * QCH)
                e_tiles = [None] * kt

                def emit_score(kti):
                    sps = ps_mm.tile([128, QCH], F32, tag="mm")
                    nc.tensor.matmul(
                        sps,
                        k_t[:, h, kti * 128 : (kti + 1) * 128],
                        q_t[:, h, qsl],
                        start=True,
                        stop=True,
                    )
                    e_sb = epool.tile([128, QCH], BF16, tag="E")
                    nc.scalar.activation(e_sb, sps, EXP, scale=SCALE)
                    e_tiles[kti] = e_sb

                for kti in range(S_LOOK):
                    emit_score(kti)
                pv = ps_pv.tile([128, QCH], F32, tag="pv")
                dnp = ps_dn.tile([128, QCH], F32, tag="dn")
                # exp tiles pair+quad-summed on the DVE (independent bf16
                # adds run ~420ns) so the PE only runs 4 denominator
                # matmuls per query chunk
                npair = kt // 2
                nquad = npair // 2
                pairs = [None] * npair
                quads = [None] * nquad

                def emit_dn(i):
                    nc.tensor.matmul(
                        dnp,
                        ones_sb,
                        quads[i],
                        start=(i == 0),
                        stop=(i == nquad - 1),
                    )

                for kti in range(kt):
                    nc.tensor.matmul(
                        pv,
                        v_sb[bi][:, kti, h, :],
                        e_tiles[kti],
                        start=(kti == 0),
                        stop=(kti == kt - 1),
                    )
                    if kti % 2 == 1:
                        i = kti // 2
                        pr = prpool.tile([128, QCH], BF16, tag="pr")
                        nc.vector.tensor_add(pr, e_tiles[kti - 1], e_tiles[kti])
                        pairs[i] = pr
                        if i % 2 == 1:
                            qi = i // 2
                            qd = prpool.tile([128, QCH], BF16, tag="qd")
                            nc.vector.tensor_add(qd, pairs[i - 1], pairs[i])
                            quads[qi] = qd
                            if qi >= 2:
                                emit_dn(qi - 2)
                    if kti + S_LOOK < kt:
                        emit_score(kti + S_LOOK)
                emit_dn(nquad - 2)
                emit_dn(nquad - 1)
                # dnp holds the softmax denominator replicated on every
                # partition: reciprocal + normalize fuse into two DVE ops
                rec = s2pool.tile([128, QCH], F32, tag="rec")
                nc.vector.reciprocal_approx_fast(rec, dnp)
                nc.vector.tensor_mul(attn_sb[bi][:, h, qsl], pv, rec)

            def s3_block(bi, c4):
                """Out-projection partial for 512 tokens of batch bi."""
                off = c4 * TCH
                gsl = slice(bi * t + off, bi * t + off + TCH)
                osb = s3pool.tile([128, dt_, TCH], FP16, tag="o")
                for do in range(dt_):
                    pool_, ptag = (ps_mm, "mm") if do % 2 == 0 else (ps_pv, "pv")
                    ps = pool_.tile([128, TCH], F32, tag=ptag)
                    for ft in range(HPC):
                        nc.tensor.matmul(
                            ps,
                            wo_sb[:, ft, do * 128 : (do + 1) * 128],
                            attn_sb[bi][:, ft, off : off + TCH],
                            start=(ft == 0),
                            stop=(ft == HPC - 1),
                        )
                    if do % 2 == 0:
                        nc.scalar.copy(osb[:, do, :], ps)
                    else:
                        nc.vector.tensor_copy(osb[:, do, :], ps)
                # one batched DMA per 512-token block (16 row-blocks); the
                # final block splits in four so the drain overlaps the
                # copies instead of trailing the kernel
                ring = nc.sync if c4 % 2 == 0 else nc.scalar
                dst = outP[:, gsl].rearrange("(do p) tt -> p do tt", p=128)
                if bi == b - 1 and c4 == cpb - 1:
                    for d0 in range(0, dt_, 4):
                        ring.dma_start(
                            out=dst[:, d0 : d0 + 4, :],
                            in_=osb[:, d0 : d0 + 4, :],
                        )
                else:
                    ring.dma_start(out=dst, in_=osb)

            # ================= riffled emission ========================
            s1_load(0, 0)
            s1_load(0, 1)
            for c in range(cpb):
                s1_chunk(0, c)
                if c + 2 < cpb:
                    s1_load(0, c + 2)
            for bi in range(b):
                plan = [
                    ("s1l", bi + 1, 0),
                    ("s2", bi, 0, 0), ("s1l", bi + 1, 1), ("s2", bi, 0, 1),
                    ("s1", bi + 1, 0),
                    ("s2", bi, 0, 2), ("s3", bi - 1, 0), ("s2", bi, 0, 3),
                    ("s1", bi + 1, 1), ("s1l", bi + 1, 2),
                    ("s2", bi, 1, 0), ("s3", bi - 1, 1), ("s2", bi, 1, 1),
                    ("s1", bi + 1, 2), ("s1l", bi + 1, 3),
                    ("s2", bi, 1, 2), ("s3", bi - 1, 2),
                    ("s2", bi, 1, 3), ("s1", bi + 1, 3),
                    ("s3", bi - 1, 3),
                ]
                for unit in plan:
                    kind = unit[0]
                    if kind == "s1l" and unit[1] < b:
                        s1_load(unit[1], unit[2])
                    elif kind == "s1" and unit[1] < b:
                        s1_chunk(unit[1], unit[2])
                    elif kind == "s2":
                        s2_block(unit[1], unit[2], unit[3])
                    elif kind == "s3" and unit[1] >= 0:
                        s3_block(unit[1], unit[2])
            for c4 in range(cpb):
                s3_block(b - 1, c4)

    nc.finalize()
    return nc


_module_cache = {}


def _get_module(b, t, d_model, n_cores):
    key = (b, t, d_model, n_cores)
    if key not in _module_cache:
        _module_cache[key] = build_module(b, t, d_model, n_cores)
    return _module_cache[key]


def _host_tables(t):
    half = HEAD_DIM // 2
    theta = 1.0 / (
        np.float32(ROPE_BASE)
        ** (np.arange(half, dtype=np.float32) / np.float32(half))
    )
    freqs = np.arange(t, dtype=np.float32)[:, None] * theta[None, :]
    emb = np.concatenate([freqs, freqs], axis=-1)  # (t, 128)
    cosT = np.ascontiguousarray(np.cos(emb).T.astype(np.float32))
    sinT = np.sin(emb).T.astype(np.float32)
    rsinT = sinT.copy()
    rsinT[:half] = -sinT[:half]
    rsinT = np.ascontiguousarray(rsinT)
    return cosT, rsinT


def _run(x, Wq, Wk, Wv, Wo, trace=False):
    b_, t_, d_ = x.shape
    n_cores = (d_ // HEAD_DIM) // HPC
    nc = _get_module(b_, t_, d_, n_cores)

    bf16 = ml_dtypes.bfloat16
    xT = np.ascontiguousarray(x.reshape(b_ * t_, d_).T.astype(bf16))
    cosT, rsinT = _host_tables(t_)
    onesc = np.ones((128, 128), dtype=bf16)

    in_maps = []
    for c in range(n_cores):
        fs = slice(c * F_LOC, (c + 1) * F_LOC)
        in_maps.append(
            {
                "xT": xT,
                "wqT": np.ascontiguousarray(Wq[fs, :].T.astype(bf16)),
                "wkT": np.ascontiguousarray(Wk[fs, :].T.astype(bf16)),
                "wvT": np.ascontiguousarray(Wv[fs, :].T.astype(bf16)),
                "woT": np.ascontiguousarray(Wo[:, fs].T.astype(bf16)),
                "cosT": cosT,
                "rsinT": rsinT,
                "onesc": onesc,
            }
        )
    res = run_bass_kernel_spmd(
        nc, in_maps, core_ids=list(range(n_cores)), trace=trace
    )
    acc = res.results[0]["outP"].astype(np.float32)
    for c in range(1, n_cores):
        acc += res.results[c]["outP"].astype(np.float32)
    out = np.ascontiguousarray(acc.T).reshape(b_, t_, d_)
    return out, res


def kernel(x, Wq, Wk, Wv, Wo):
    x = np.asarray(x, dtype=np.float32)
    Wq = np.asarray(Wq, dtype=np.float32)
    Wk = np.asarray(Wk, dtype=np.float32)
    Wv = np.asarray(Wv, dtype=np.float32)
    Wo = np.asarray(Wo, dtype=np.float32)
    out, _ = _run(x, Wq, Wk, Wv, Wo, trace=False)
    return out


# revision 58
# speedup vs baseline: 1.0041x; 1.0041x over previous
"""Trainium2 Bass kernel: 16-head RoPE attention block (B=4, T=2048, D=2048).

Sharding: tensor-parallel over heads. Each of the 8 cores owns 2 heads
(a 256-wide slice of the q/k/v projection output features) and computes a
full-width partial of the output projection; the host sums the 8 fp16
partials (the "all-reduce").

v2 structure (vs the DRAM-scratch baseline):
  - everything flows in bf16 (weights, x, q/k/v, exp(S), attention, Wo);
    PSUM accumulation stays fp32. Halves DMA + SBUF and keeps the PE at
    1 col/cycle.
  - q/k/v live in SBUF per batch (double-buffered) - no DRAM round-trip.
  - per-batch software pipeline, riffled into one instruction stream so
    the PE queue interleaves stage-2 attention blocks of batch i with
    stage-1 projection chunks of batch i+1 and stage-3 out-projection
    blocks of batch i-1: the activation engine's exp backlog drains
    during the projection/out-proj blocks instead of stalling PV.
  - softmax denominators use a 128-wide ones matrix as the stationary
    operand, so the PE replicates sum(exp) across all partitions: the
    broadcast is free, and reciprocal+normalize fuse into two DVE ops
    per query chunk (no DRAM round-trip for the denominators).
  - outputs written as fp16 partials (halves the output DMA).
"""

import math

import numpy as np
import ml_dtypes

import concourse.bacc as bacc
import concourse.bass as bass
import concourse.mybir as mybir
import concourse.tile as tile
from concourse.bass_utils import run_bass_kernel_spmd

F32 = mybir.dt.float32
BF16 = mybir.dt.bfloat16
FP16 = mybir.dt.float16
EXP = mybir.ActivationFunctionType.Exp

# Problem shape (hardcoded; the harness calls kernel() with exactly these).
B = 4
T = 2048
D_MODEL = 2048
HEAD_DIM = 128
N_CORES = 8
ROPE_BASE = 10000.0

HPC = 2                      # heads per core
F_LOC = HPC * HEAD_DIM       # 256 local projection features per core
TCH = 512                    # stage-1 token chunk width
QCH = 512                    # stage-2 query chunk width
SCALE = 1.0 / math.sqrt(HEAD_DIM)
S_LOOK = 5                   # score-matmul lookahead in the attention loop


def build_module(b=B, t=T, d_model=D_MODEL, n_cores=N_CORES):
    """Build the per-core Bass module. All cores run the same program on
    different data (pure SPMD, no collectives)."""
    dt_ = d_model // 128     # 16 contraction tiles
    kt = t // 128            # 16 key tiles per batch
    cpb = t // TCH           # 4 stage-1 chunks per batch
    nqc = t // QCH           # 4 query chunks

    nc = bacc.Bacc(None, target_bir_lowering=False)

    xT = nc.dram_tensor("xT", [d_model, b * t], BF16, kind="ExternalInput")
    wqT = nc.dram_tensor("wqT", [d_model, F_LOC], BF16, kind="ExternalInput")
    wkT = nc.dram_tensor("wkT", [d_model, F_LOC], BF16, kind="ExternalInput")
    wvT = nc.dram_tensor("wvT", [d_model, F_LOC], BF16, kind="ExternalInput")
    woT = nc.dram_tensor("woT", [F_LOC, d_model], BF16, kind="ExternalInput")
    cosT = nc.dram_tensor("cosT", [HEAD_DIM, t], F32, kind="ExternalInput")
    rsinT = nc.dram_tensor("rsinT", [HEAD_DIM, t], F32, kind="ExternalInput")
    onesc = nc.dram_tensor("onesc", [128, 128], BF16, kind="ExternalInput")
    outP = nc.dram_tensor("outP", [d_model, b * t], FP16, kind="ExternalOutput")

    with tile.TileContext(nc) as tc:
        with (
            tc.tile_pool(name="const", bufs=1) as constp,
            tc.tile_pool(name="dram", bufs=1, space="DRAM") as dram,
            tc.tile_pool(name="wq", bufs=1) as wpool,
            tc.tile_pool(name="x", bufs=2) as xpool,
            tc.tile_pool(name="qkv", bufs=2) as qkvp,
            tc.tile_pool(name="t1", bufs=3) as tpool,
            tc.tile_pool(name="e", bufs=8) as epool,
            tc.tile_pool(name="pr", bufs=4) as prpool,
            tc.tile_pool(name="s2", bufs=2) as s2pool,
            tc.tile_pool(name="attn", bufs=2) as attnp,
            tc.tile_pool(name="s3o", bufs=2) as s3pool,
            tc.tile_pool(name="ps_mm", bufs=5, space="PSUM") as ps_mm,
            tc.tile_pool(name="ps_pv", bufs=2, space="PSUM") as ps_pv,
            tc.tile_pool(name="ps_dn", bufs=1, space="PSUM") as ps_dn,
        ):
            # ---- constants (gpsimd ring keeps sync/scalar free for the
            # weight/x loads that gate the first matmuls) ----
            cos_sb = constp.tile([128, t], BF16)
            nc.gpsimd.dma_start(out=cos_sb, in_=cosT[:, :])
            rsin_sb = constp.tile([128, t], BF16)
            nc.gpsimd.dma_start(out=rsin_sb, in_=rsinT[:, :])


            # 128-wide ones: the denominator matmul replicates sum(exp)
            # across all 128 output partitions, making the broadcast free.
            # Loaded first (tiny, sync ring) so the PE warm-up matmuls can
            # ramp the p-state while the weight/x DMAs land.
            ones_sb = constp.tile([128, 128], BF16)
            nc.sync.dma_start(out=ones_sb, in_=onesc[:, :])
            warm_ps = ps_dn.tile([128, QCH], F32, tag="dn")
            for wu in range(36):
                nc.tensor.matmul(
                    warm_ps[:, 0:128], ones_sb, ones_sb, start=True, stop=True
                )

            # ---- weights ----
            w_sbs = []
            for wi, (wten, wname) in enumerate(
                ((wqT, "wq"), (wkT, "wk"), (wvT, "wv"))
            ):
                wsb = wpool.tile([128, dt_, F_LOC], BF16, tag=wname)
                src = wten[:, :].rearrange("(dt p) f -> p dt f", p=128)
                # wq on the sync ring ahead of x chunk 0, split so the
                # first projections start after 1/4 of the transfer;
                # wk/wv on the scalar HWDGE ring
                if wi == 0:
                    for d0 in range(0, dt_, 4):
                        nc.sync.dma_start(
                            out=wsb[:, d0 : d0 + 4, :],
                            in_=src[:, d0 : d0 + 4, :],
                        )
                else:
                    nc.scalar.dma_start(out=wsb, in_=src)
                w_sbs.append(wsb)
            wo_sb = wpool.tile([128, HPC, d_model], BF16, tag="wo")
            nc.scalar.dma_start(
                out=wo_sb,
                in_=woT[:, :].rearrange("(ft p) d -> p ft d", p=128),
            )

            # per-batch double-buffered SBUF state, created lazily
            qk_sb = {}       # (bi,) -> (q_sb, k_sb)  [128, HPC, t] bf16
            v_sb = {}        # (bi,) -> [128, kt, HPC, 128] bf16
            attn_sb = {}     # (bi,) -> [128, HPC, t] bf16

            # ================= emission units =========================
            x_tiles = {}

            def s1_load(bi, c):
                """Issue the x-chunk DMA (placed ahead of its compute)."""
                off = c * TCH
                tsl = slice(bi * t + off, bi * t + off + TCH)
                x_sb = xpool.tile([128, dt_, TCH], BF16, name=f"x{bi}_{c}", tag="x")
                xsrc = xT[:, tsl].rearrange("(dt p) tt -> p dt tt", p=128)
                if bi == 0 and c == 0:
                    # split the first chunk's load so the very first
                    # matmuls start after 1/4 of the transfer
                    for d0 in range(0, dt_, 4):
                        nc.sync.dma_start(
                            out=x_sb[:, d0 : d0 + 4, :],
                            in_=xsrc[:, d0 : d0 + 4, :],
                        )
                else:
                    nc.sync.dma_start(out=x_sb, in_=xsrc)
                x_tiles[(bi, c)] = x_sb

            def s1_chunk(bi, c):
                """Projections + rope + v-transpose for 512 tokens."""
                if c == 0:
                    qk_sb[bi] = (
                        qkvp.tile([128, HPC, t], BF16, name=f"q{bi}", tag="q"),
                        qkvp.tile([128, HPC, t], BF16, name=f"k{bi}", tag="k"),
                    )
                    v_sb[bi] = qkvp.tile(
                        [128, kt, HPC, 128], BF16, name=f"v{bi}", tag="v"
                    )
                off = c * TCH
                lsl = slice(off, off + TCH)
                x_sb = x_tiles.pop((bi, c))
                for pi in range(3):
                    for ft in range(HPC):
                        fsl = slice(ft * 128, (ft + 1) * 128)
                        ps = ps_mm.tile([128, TCH], F32, tag="mm")
                        for di in range(dt_):
                            nc.tensor.matmul(
                                ps,
                                w_sbs[pi][:, di, fsl],
                                x_sb[:, di, :],
                                start=(di == 0),
                                stop=(di == dt_ - 1),
                            )
                        if pi < 2:
                            # rope: out = in*cos + rot_half(in)*sin
                            ro = tpool.tile([128, TCH], F32, tag="ro")
                            nc.vector.tensor_mul(ro, ps, cos_sb[:, lsl])
                            rt = tpool.tile([128, TCH], F32, tag="rt")
                            nc.vector.tensor_mul(
                                rt[0:64], ps[64:128], rsin_sb[0:64, lsl]
                            )
                            nc.vector.tensor_mul(
                                rt[64:128], ps[0:64], rsin_sb[64:128, lsl]
                            )
                            dst = qk_sb[bi][pi]
                            nc.vector.tensor_add(dst[:, ft, lsl], ro, rt)
                        else:
                            # v lands token-major via an XBAR DMA transpose
                            # per chunk (out[p, j, d] = vsb[d, j*128+p])
                            vsb = tpool.tile([128, TCH], BF16, tag="vs")
                            nc.scalar.copy(vsb, ps)
                            j0 = c * (TCH // 128)
                            nc.sync.dma_start_transpose(
                                v_sb[bi][:, j0 : j0 + TCH // 128, ft, :], vsb
                            )

            def s2_block(bi, h, qc):
                """Attention for one (batch, head, 512-query chunk)."""
                if h == 0 and qc == 0:
                    attn_sb[bi] = attnp.tile(
                        [128, HPC, t], BF16, name=f"an{bi}", tag="an"
                    )
                q_t, k_t = qk_sb[bi]
                qsl = slice(qc * QCH, (qc + 1) * QCH)
                e_tiles = [None] * kt

                def emit_score(kti):
                    sps = ps_mm.tile([128, QCH], F32, tag="mm")
                    nc.tensor.matmul(
                        sps,
                        k_t[:, h, kti * 128 : (kti + 1) * 128],
                        q_t[:, h, qsl],
                        start=True,
                        stop=True,
                    )
                    e_sb = epool.tile([128, QCH], BF16, tag="E")
                    nc.scalar.activation(e_sb, sps, EXP, scale=SCALE)
                    e_tiles[kti] = e_sb

                for kti in range(S_LOOK):
                    emit_score(kti)
                pv = ps_pv.tile([128, QCH], F32, tag="pv")
                dnp = ps_dn.tile([128, QCH], F32, tag="dn")
                # exp tiles pair+quad-summed on the DVE (independent bf16
                # adds run ~420ns) so the PE only runs 4 denominator
                # matmuls per query chunk
                npair = kt // 2
                nquad = npair // 2
                pairs = [None] * npair
                quads = [None] * nquad

                def emit_dn(i):
                    nc.tensor.matmul(
                        dnp,
                        ones_sb,
                        quads[i],
                        start=(i == 0),
                        stop=(i == nquad - 1),
                    )

                for kti in range(kt):
                    nc.tensor.matmul(
                        pv,
                        v_sb[bi][:, kti, h, :],
                        e_tiles[kti],
                        start=(kti == 0),
                        stop=(kti == kt - 1),
                    )
                    if kti % 2 == 1:
                        i = kti // 2
                        pr = prpool.tile([128, QCH], BF16, tag="pr")
                        nc.vector.tensor_add(pr, e_tiles[kti - 1], e_tiles[kti])
                        pairs[i] = pr
                        if i % 2 == 1:
                            qi = i // 2
                            qd = prpool.tile([128, QCH], BF16, tag="qd")
                            nc.vector.tensor_add(qd, pairs[i - 1], pairs[i])
                            quads[qi] = qd
                            if qi >= 2:
                                emit_dn(qi - 2)
                    if kti + S_LOOK < kt:
                        emit_score(kti + S_LOOK)
                emit_dn(nquad - 2)
                emit_dn(nquad - 1)
                # dnp holds the softmax denominator replicated on every
                # partition: reciprocal + normalize fuse into two DVE ops
                rec = s2pool.tile([128, QCH], F32, tag="rec")
                nc.vector.reciprocal_approx_fast(rec, dnp)
                nc.vector.tensor_mul(attn_sb[bi][:, h, qsl], pv, rec)

            def s3_block(bi, c4):
                """Out-projection partial for 512 tokens of batch bi."""
                off = c4 * TCH
                gsl = slice(bi * t + off, bi * t + off + TCH)
                osb = s3pool.tile([128, dt_, TCH], FP16, tag="o")
                for do in range(dt_):
                    pool_, ptag = (ps_mm, "mm") if do % 2 == 0 else (ps_pv, "pv")
                    ps = pool_.tile([128, TCH], F32, tag=ptag)
                    for ft in range(HPC):
                        nc.tensor.matmul(
                            ps,
                            wo_sb[:, ft, do * 128 : (do + 1) * 128],
                            attn_sb[bi][:, ft, off : off + TCH],
                            start=(ft == 0),
                            stop=(ft == HPC - 1),
                        )
                    if do % 2 == 0:
                        nc.scalar.copy(osb[:, do, :], ps)
                    else:
                        nc.vector.tensor_copy(osb[:, do, :], ps)
                # one batched DMA per 512-token block (16 row-blocks); the
                # final block splits in four so the drain overlaps the
                # copies instead of trailing the kernel
                ring = nc.sync if c4 % 2 == 0 else nc.scalar
                dst = outP[:, gsl].rearrange("(do p) tt -> p do tt", p=128)
                if bi == b - 1 and c4 == cpb - 1:
                    for d0 in range(0, dt_, 4):
                        ring.dma_start(
                            out=dst[:, d0 : d0 + 4, :],
                            in_=osb[:, d0 : d0 + 4, :],
                        )
                else:
                    ring.dma_start(out=dst, in_=osb)

            # ================= riffled emission ========================
            s1_load(0, 0)
            s1_load(0, 1)
            for c in range(cpb):
                s1_chunk(0, c)
                if c + 2 < cpb:
                    s1_load(0, c + 2)
            for bi in range(b):
                plan = [
                    ("s1l", bi + 1, 0),
                    ("s2", bi, 0, 0), ("s1l", bi + 1, 1), ("s2", bi, 0, 1),
                    ("s1", bi + 1, 0),
                    ("s2", bi, 0, 2), ("s3", bi - 1, 0), ("s2", bi, 0, 3),
                    ("s1", bi + 1, 1), ("s1l", bi + 1, 2),
                    ("s2", bi, 1, 0), ("s3", bi - 1, 1), ("s2", bi, 1, 1),
                    ("s1", bi + 1, 2), ("s1l", bi + 1, 3),
                    ("s2", bi, 1, 2), ("s3", bi - 1, 2),
                    ("s2", bi, 1, 3), ("s1", bi + 1, 3),
                    ("s3", bi - 1, 3),
                ]
                for unit in plan:
                    kind = unit[0]
                    if kind == "s1l" and unit[1] < b:
                        s1_load(unit[1], unit[2])
                    elif kind == "s1" and unit[1] < b:
                        s1_chunk(unit[1], unit[2])
                    elif kind == "s2":
                        s2_block(unit[1], unit[2], unit[3])
                    elif kind == "s3" and unit[1] >= 0:
                        s3_block(unit[1], unit[2])
            for c4 in range(cpb):
                s3_block(b - 1, c4)

    nc.finalize()
    return nc


_module_cache = {}


def _get_module(b, t, d_model, n_cores):
    key = (b, t, d_model, n_cores)
    if key not in _module_cache:
        _module_cache[key] = build_module(b, t, d_model, n_cores)
    return _module_cache[key]


def _host_tables(t):
    half = HEAD_DIM // 2
    theta = 1.0 / (
        np.float32(ROPE_BASE)
        ** (np.arange(half, dtype=np.float32) / np.float32(half))
    )
    freqs = np.arange(t, dtype=np.float32)[:, None] * theta[None, :]
    emb = np.concatenate([freqs, freqs], axis=-1)  # (t, 128)
    cosT = np.ascontiguousarray(np.cos(emb).T.astype(np.float32))
    sinT = np.sin(emb).T.astype(np.float32)
    rsinT = sinT.copy()
    rsinT[:half] = -sinT[:half]
    rsinT = np.ascontiguousarray(rsinT)
    return cosT, rsinT


def _run(x, Wq, Wk, Wv, Wo, trace=False):
    b_, t_, d_ = x.shape
    n_cores = (d_ // HEAD_DIM) // HPC
    nc = _get_module(b_, t_, d_, n_cores)

    bf16 = ml_dtypes.bfloat16
    xT = np.ascontiguousarray(x.reshape(b_ * t_, d_).T.astype(bf16))
    cosT, rsinT = _host_tables(t_)
    onesc = np.ones((128, 128), dtype=bf16)

    in_maps = []
    for c in range(n_cores):
        fs = slice(c * F_LOC, (c + 1) * F_LOC)
        in_maps.append(
            {
                "xT": xT,
                "wqT": np.ascontiguousarray(Wq[fs, :].T.astype(bf16)),
                "wkT": np.ascontiguousarray(Wk[fs, :].T.astype(bf16)),
                "wvT": np.ascontiguousarray(Wv[fs, :].T.astype(bf16)),
                "woT": np.ascontiguousarray(Wo[:, fs].T.astype(bf16)),
                "cosT": cosT,
                "rsinT": rsinT,
                "onesc": onesc,
            }
        )
    res = run_bass_kernel_spmd(
        nc, in_maps, core_ids=list(range(n_cores)), trace=trace
    )
    acc = res.results[0]["outP"].astype(np.float32)
    for c in range(1, n_cores):
        acc += res.results[c]["outP"].astype(np.float32)
    out = np.ascontiguousarray(acc.T).reshape(b_, t_, d_)
    return out, res


def kernel(x, Wq, Wk, Wv, Wo):
    x = np.asarray(x, dtype=np.float32)
    Wq = np.asarray(Wq, dtype=np.float32)
    Wk = np.asarray(Wk, dtype=np.float32)
    Wv = np.asarray(Wv, dtype=np.float32)
    Wo = np.asarray(Wo, dtype=np.float32)
    out, _ = _run(x, Wq, Wk, Wv, Wo, trace=False)
    return out
